# revision 9
# baseline (speedup 1.0000x reference)
"""Trainium2 Bass kernel for nn_MiniQuixerTQ (MiniQuixer with 10-qubit PQC).

Math used to restructure the reference:
  - Only the v-projection of qkv is consumed, and only through a mean over T:
      context = mean_T(h) @ w_v + b_v,  mean_T(h) = emb_sum/T + mean(pos_emb)
    so fold W_a = w_v @ in_w, const_a = (pos_mean @ W_a) + b_v @ in_w + in_b.
  - Only the last token flows through LN1/FFN/LN2/logits.
  - PQC: the encoded state after the data-encoding RX layer is a product
    state built from cos/sin half-angles; the variational layers + CNOT
    chains form a fixed (input-independent) unitary U computed on host from
    pqc_params; device applies psi_out = psi_enc @ U^T as real matmuls.
  - z @ map_w = probs @ (S @ map_w) with S[i,q] = 1-2*bit_q(i); map_b and
    pos_emb[-1] folded in via a K=1 augmentation row.

Sharding (8 cores): T-sharded gather (64 positions x 32 samples each) ->
AllReduce emb_sum -> replicated tiny PQC/LN1 path -> column-sharded FFN
partial -> AllReduce -> replicated LN2 -> vocab-sharded logits [32,4000].
"""
import sys

sys.path.insert(0, "/opt/trn_rl_repo")

import numpy as np
import concourse.bass as bass
import concourse.mybir as mybir
import concourse.tile as tile
from concourse.bass_utils import run_bass_kernel_spmd
from concourse.masks import make_identity

B, T, E, V, Q, L = 32, 512, 1024, 32000, 10, 8
DQKV = E // 8
NC = 8
TS = T // NC        # 64 positions per core
VS = V // NC        # 4000 vocab per core
FH = 4 * E // NC    # 512 ffn hidden per core
DIM = 2 ** Q        # 1024 statevector
KC = E // 128       # 8 K-chunks of 128
f32 = mybir.dt.float32
i32 = mybir.dt.int32

TRACE = False
_last = {}

WAIT_LIMIT = 1


def _split_excess_waits(nc, limit=WAIT_LIMIT):
    """walrus(CoreV2/V3) rejects >limit sync waits per instruction; move the
    excess onto nofuse nops inserted right before, on the same engine."""
    for fn in nc.m.functions:
        for bb in fn.blocks:
            insts = bb.instructions
            i = 0
            while i < len(insts):
                ins = insts[i]
                si = getattr(ins, "sync_info", None)
                if si is None or not si.on_wait or len(si.on_wait) <= limit:
                    i += 1
                    continue
                waits = list(si.on_wait)
                si.on_wait = waits[:limit]
                extra = waits[limit:]
                nops = []
                for j in range(0, len(extra), limit):
                    nop_ins = nc.engines[ins.engine].nop(
                        nofuse=True, hint="waitsplit"
                    ).ins
                    cur = nc.cur_bb.bb
                    assert cur.instructions[-1] is nop_ins
                    cur.instructions.pop()
                    if nop_ins.sync_info is None:
                        nop_ins.sync_info = mybir.SyncInfo(
                            on_wait=extra[j : j + limit], on_update=[]
                        )
                    else:
                        nop_ins.sync_info.on_wait = extra[j : j + limit]
                    nops.append(nop_ins)
                for k, nop_ins in enumerate(nops):
                    insts.insert(i + k, nop_ins)
                i += len(nops) + 1


# ---------------------------------------------------------------- host math
def _rx(t):
    c, s = np.cos(t / 2), np.sin(t / 2)
    return np.array([[c, -1j * s], [-1j * s, c]], np.complex128)


def _ry(t):
    c, s = np.cos(t / 2), np.sin(t / 2)
    return np.array([[c, -s], [s, c]], np.complex128)


def _rz(t):
    e = np.exp(-1j * t / 2)
    return np.array([[e, 0], [0, np.conj(e)]], np.complex128)


def _apply_gate_rows(M, g, q):
    # M: [rows, DIM] batch of row-states; gate on wire q (wire0 = MSB)
    s = M.reshape(M.shape[0], 2 ** q, 2, -1)
    a0 = s[:, :, 0]
    a1 = s[:, :, 1]
    n0 = g[0, 0] * a0 + g[0, 1] * a1
    n1 = g[1, 0] * a0 + g[1, 1] * a1
    out = np.stack([n0, n1], axis=2)
    return out.reshape(M.shape[0], DIM)


def _build_ut(params):
    """Row-convention transfer matrix of the variational circuit:
    psi_out_row = psi_enc_row @ UT."""
    M = np.eye(DIM, dtype=np.complex128)
    for l in range(L):
        for q in range(Q):
            M = _apply_gate_rows(M, _rx(params[l, q, 0]), q)
            M = _apply_gate_rows(M, _ry(params[l, q, 1]), q)
            M = _apply_gate_rows(M, _rz(params[l, q, 2]), q)
        for q in range(Q - 1):
            s = M.reshape(DIM, 2 ** q, 2, 2, -1).copy()
            s[:, :, 1] = s[:, :, 1, ::-1]
            M = s.reshape(DIM, DIM)
    return M


# ----------------------------------------------------------- device program
def _build_program():
    nc = bass.Bass(trn_type="TRN2")

    d = {}

    def di(name, shape, dtype=f32):
        d[name] = nc.dram_tensor(name, shape, dtype, kind="ExternalInput")
        return d[name]

    tok_emb = di("tok_emb", [V, E])
    tembt = di("tembt", [E, VS])          # tok_emb[vshard].T
    idx_t = di("idx_t", [B * TS, 1], i32)  # sample-major, 64 per sample
    idx_l = di("idx_l", [B, 1], i32)
    sel = di("sel", [128, 16 * 32])        # selector for gather-sum matmuls
    wa = di("wa", [E, Q])
    ca_rep = di("ca_rep", [B, Q])
    masks = di("masks", [128, 3 * KC])     # mrT|miT|miNT chunk-major
    utr = di("utr", [DIM, DIM])
    uti = di("uti", [DIM, DIM])
    sm = di("sm", [DIM, E])                # S @ map_w
    sm_last = di("sm_last", [1, E])        # map_b + pos_emb[-1]
    g1r = di("g1r", [B, E])
    b1r = di("b1r", [B, E])
    g2r = di("g2r", [B, E])
    b2r = di("b2r", [B, E])
    b1f_rep = di("b1f_rep", [B, FH])       # ffn_b1 shard replicated
    b2f_rep = di("b2f_rep", [B, E])        # full ffn_b2 replicated
    w1k = di("w1k", [E, FH])               # ffn_w1[:, shard]
    w2k = di("w2k", [FH, E])               # ffn_w2[shard, :]

    logits = nc.dram_tensor("logits", [B, VS], f32, kind="ExternalOutput")

    AT = mybir.AluOpType
    AF = mybir.ActivationFunctionType
    AX = mybir.AxisListType

    with tile.TileContext(nc) as tc:
        with (
            tc.tile_pool(name="stat", bufs=1) as st,
            tc.tile_pool(name="gath", bufs=3) as gp,
            tc.tile_pool(name="strm", bufs=6) as sp,
            tc.tile_pool(name="work", bufs=1) as wp,
            tc.tile_pool(name="lgp", bufs=3) as lgp,
            tc.tile_pool(name="pbig", bufs=2, space="PSUM") as pb,
            tc.tile_pool(name="pone", bufs=2, space="PSUM") as po,
            tc.tile_pool(name="plog", bufs=2, space="PSUM") as pl,
            tc.tile_pool(name="dram", bufs=1, space="DRAM") as dp,
        ):
            ident = st.tile([128, 128], f32)
            make_identity(nc, ident[:])

            def transpose_1024(src32, dst128t):
                """src32 [32, 1024] sbuf -> dst128t [128, KC*32] sbuf."""
                for c in range(KC):
                    pt = po.tile([128, 32], f32, tag="ptr", space="PSUM")
                    nc.tensor.transpose(
                        out=pt[:, :],
                        in_=src32[:, 128 * c : 128 * (c + 1)],
                        identity=ident[:32, :32],
                    )
                    nc.vector.tensor_copy(
                        dst128t[:, 32 * c : 32 * (c + 1)], pt[:, :]
                    )

            # ---- phase A: gather + per-sample sums (T-shard) -------------
            sel_t = st.tile([128, 16 * 32], f32)
            nc.sync.dma_start(out=sel_t[:], in_=sel[:])
            ps_emb = pb.tile([32, 1024], f32, tag="big", space="PSUM")
            for j in range(16):
                ix = gp.tile([128, 1], i32, tag="ix")
                nc.sync.dma_start(out=ix[:], in_=idx_t[128 * j : 128 * (j + 1), :])
                rows = gp.tile([128, 1024], f32, tag="rows")
                nc.gpsimd.indirect_dma_start(
                    out=rows[:],
                    out_offset=None,
                    in_=tok_emb[:],
                    in_offset=bass.IndirectOffsetOnAxis(ap=ix[:, :1], axis=0),
                )
                for h in range(2):
                    nc.tensor.matmul(
                        ps_emb[:, 512 * h : 512 * (h + 1)],
                        lhsT=sel_t[:, 32 * j : 32 * (j + 1)],
                        rhs=rows[:, 512 * h : 512 * (h + 1)],
                        start=(j == 0),
                        stop=(j == 15),
                    )

            # last-token rows (replicated)
            ixl = st.tile([B, 1], i32)
            nc.sync.dma_start(out=ixl[:], in_=idx_l[:])
            hlast = st.tile([B, 1024], f32)
            nc.gpsimd.indirect_dma_start(
                out=hlast[:],
                out_offset=None,
                in_=tok_emb[:],
                in_offset=bass.IndirectOffsetOnAxis(ap=ixl[:, :1], axis=0),
            )

            # ---- AllReduce #1: emb_sum ----------------------------------
            emb_s = wp.tile([32, 1024], f32)
            nc.vector.tensor_copy(emb_s[:], ps_emb[:])
            cc1i = dp.tile([32, 1024], f32)
            cc1o = dp.tile([32, 1024], f32)
            nc.sync.dma_start(out=cc1i[:], in_=emb_s[:])
            nc.gpsimd.collective_compute(
                "AllReduce",
                AT.add,
                replica_groups=[list(range(NC))],
                ins=[cc1i.opt()],
                outs=[cc1o.opt()],
            )
            sum_all = st.tile([32, 1024], f32)
            nc.sync.dma_start(out=sum_all[:], in_=cc1o[:])

            # ---- angles --------------------------------------------------
            sum_t = st.tile([128, KC * 32], f32)
            transpose_1024(sum_all, sum_t)
            wa_t = st.tile([128, KC * Q], f32)
            for c in range(KC):
                nc.sync.dma_start(
                    out=wa_t[:, Q * c : Q * (c + 1)],
                    in_=wa[128 * c : 128 * (c + 1), :],
                )
            ps_ang = po.tile([32, Q], f32, tag="ptr", space="PSUM")
            for c in range(KC):
                nc.tensor.matmul(
                    ps_ang[:, :],
                    lhsT=sum_t[:, 32 * c : 32 * (c + 1)],
                    rhs=wa_t[:, Q * c : Q * (c + 1)],
                    start=(c == 0),
                    stop=(c == KC - 1),
                )
            ca_t = st.tile([B, Q], f32)
            nc.sync.dma_start(out=ca_t[:], in_=ca_rep[:])
            ang0 = st.tile([32, Q], f32)
            nc.scalar.mul(ang0[:], ps_ang[:], 1.0 / T)
            ang1 = st.tile([32, Q], f32)
            nc.vector.tensor_tensor(out=ang1[:], in0=ang0[:], in1=ca_t[:], op=AT.add)
            th = st.tile([32, Q], f32)   # pi*tanh(.)/2
            nc.scalar.activation(th[:], ang1[:], AF.Tanh)
            nc.scalar.mul(th[:], th[:], float(np.pi / 2))
            halfpi = st.tile([32, 1], f32)
            nc.vector.memset(halfpi[:], float(np.pi / 2))
            c_t = st.tile([32, Q], f32)
            nc.scalar.activation(c_t[:], th[:], AF.Sin, bias=halfpi[:, :1])
            s_t = st.tile([32, Q], f32)
            nc.scalar.activation(s_t[:], th[:], AF.Sin)

            # ---- product state -------------------------------------------
            pa = st.tile([32, 1024], f32)
            pbuf = st.tile([32, 1024], f32)
            nc.vector.memset(pa[:, :1], 1.0)
            cur, nxt = pa, pbuf
            for k in range(Q):
                w = Q - 1 - k
                n = 1 << k
                nc.vector.tensor_scalar_mul(
                    out=nxt[:, :n], in0=cur[:, :n], scalar1=c_t[:, w : w + 1]
                )
                nc.vector.tensor_scalar_mul(
                    out=nxt[:, n : 2 * n], in0=cur[:, :n], scalar1=s_t[:, w : w + 1]
                )
                cur, nxt = nxt, cur
            # cur holds p [32, 1024]
            p_t = st.tile([128, KC * 32], f32)
            transpose_1024(cur, p_t)
            msk = st.tile([128, 3 * KC], f32)
            nc.sync.dma_start(out=msk[:], in_=masks[:])
            re_t = st.tile([128, KC * 32], f32)
            im_t = st.tile([128, KC * 32], f32)
            imn_t = st.tile([128, KC * 32], f32)
            for c in range(KC):
                sl = slice(32 * c, 32 * (c + 1))
                nc.vector.tensor_scalar_mul(
                    out=re_t[:, sl], in0=p_t[:, sl], scalar1=msk[:, c : c + 1]
                )
                nc.vector.tensor_scalar_mul(
                    out=im_t[:, sl], in0=p_t[:, sl], scalar1=msk[:, KC + c : KC + c + 1]
                )
                nc.vector.tensor_scalar_mul(
                    out=imn_t[:, sl],
                    in0=p_t[:, sl],
                    scalar1=msk[:, 2 * KC + c : 2 * KC + c + 1],
                )

            # ---- psi_out = psi_enc @ UT (complex, via 4 real sums) -------
            probs = wp.tile([32, 1024], f32)
            for h in range(2):
                cs = slice(512 * h, 512 * (h + 1))
                ps_re = pb.tile([32, 512], f32, tag="big", space="PSUM")
                ps_im = pb.tile([32, 512], f32, tag="big", space="PSUM")
                for c in range(KC):
                    ur = sp.tile([128, 512], f32, tag="stream")
                    nc.sync.dma_start(out=ur[:], in_=utr[128 * c : 128 * (c + 1), cs])
                    ui = sp.tile([128, 512], f32, tag="stream")
                    nc.sync.dma_start(out=ui[:], in_=uti[128 * c : 128 * (c + 1), cs])
                    csl = slice(32 * c, 32 * (c + 1))
                    nc.tensor.matmul(
                        ps_re[:, :], lhsT=re_t[:, csl], rhs=ur[:],
                        start=(c == 0), stop=False,
                    )
                    nc.tensor.matmul(
                        ps_re[:, :], lhsT=imn_t[:, csl], rhs=ui[:],
                        start=False, stop=(c == KC - 1),
                    )
                    nc.tensor.matmul(
                        ps_im[:, :], lhsT=re_t[:, csl], rhs=ui[:],
                        start=(c == 0), stop=False,
                    )
                    nc.tensor.matmul(
                        ps_im[:, :], lhsT=im_t[:, csl], rhs=ur[:],
                        start=False, stop=(c == KC - 1),
                    )
                sq_r = wp.tile([32, 512], f32, tag="sqr")
                nc.scalar.activation(sq_r[:], ps_re[:], AF.Square)
                sq_i = wp.tile([32, 512], f32, tag="sqi")
                nc.scalar.activation(sq_i[:], ps_im[:], AF.Square)
                nc.vector.tensor_tensor(out=probs[:, cs], in0=sq_r[:], in1=sq_i[:], op=AT.add)

            # ---- pqc_exp + h_last -> ln1 input ---------------------------
            probs_t = st.tile([128, KC * 32], f32)
            transpose_1024(probs, probs_t)
            ones1 = st.tile([1, 32], f32)
            nc.vector.memset(ones1[:], 1.0)
            sml_t = st.tile([1, 1024], f32)
            nc.sync.dma_start(out=sml_t[:], in_=sm_last[:])
            ln1_in = wp.tile([32, 1024], f32)
            for h in range(2):
                cs = slice(512 * h, 512 * (h + 1))
                ps_q = pb.tile([32, 512], f32, tag="big", space="PSUM")
                for c in range(KC):
                    smt = sp.tile([128, 512], f32, tag="stream")
                    nc.sync.dma_start(out=smt[:], in_=sm[128 * c : 128 * (c + 1), cs])
                    nc.tensor.matmul(
                        ps_q[:, :],
                        lhsT=probs_t[:, 32 * c : 32 * (c + 1)],
                        rhs=smt[:],
                        start=(c == 0), stop=False,
                    )
                nc.tensor.matmul(
                    ps_q[:, :], lhsT=ones1[:], rhs=sml_t[:, cs],
                    start=False, stop=True,
                )
                nc.vector.tensor_tensor(
                    out=ln1_in[:, cs], in0=ps_q[:], in1=hlast[:, cs], op=AT.add
                )

            # ---- layernorm helper ----------------------------------------
            def layer_norm(x_in, g_rep, b_rep, out_name):
                gt = st.tile([B, E], f32, tag=out_name + "g")
                nc.sync.dma_start(out=gt[:], in_=g_rep[:])
                bt = st.tile([B, E], f32, tag=out_name + "b")
                nc.sync.dma_start(out=bt[:], in_=b_rep[:])
                s1 = st.tile([32, 1], f32, tag=out_name + "s")
                nc.vector.reduce_sum(out=s1[:], in_=x_in[:], axis=AX.X)
                m = st.tile([32, 1], f32, tag=out_name + "m")
                nc.scalar.mul(m[:], s1[:], 1.0 / E)
                cen = wp.tile([32, 1024], f32, tag="cen")
                nc.vector.tensor_scalar_sub(out=cen[:], in0=x_in[:], scalar1=m[:, :1])
                sq = wp.tile([32, 1024], f32, tag="lnsq")
                v1 = st.tile([32, 1], f32, tag=out_name + "v")
                nc.scalar.activation(sq[:], cen[:], AF.Square, accum_out=v1[:])
                eps_t = st.tile([32, 1], f32, tag=out_name + "e")
                nc.vector.memset(eps_t[:], 1e-5)
                sd = st.tile([32, 1], f32, tag=out_name + "sd")
                nc.scalar.activation(
                    sd[:], v1[:], AF.Sqrt, bias=eps_t[:, :1], scale=1.0 / E
                )
                ri = st.tile([32, 1], f32, tag=out_name + "ri")
                nc.vector.reciprocal(ri[:], sd[:])
                hn = wp.tile([32, 1024], f32, tag=out_name + "hn")
                nc.vector.tensor_scalar_mul(out=hn[:], in0=cen[:], scalar1=ri[:, :1])
                h_out = st.tile([32, 1024], f32, tag=out_name)
                nc.vector.tensor_tensor(out=h_out[:], in0=hn[:], in1=gt[:], op=AT.mult)
                nc.vector.tensor_tensor(out=h_out[:], in0=h_out[:], in1=bt[:], op=AT.add)
                return h_out

            h1 = layer_norm(ln1_in, g1r, b1r, "h1")

            # ---- FFN (column shard) --------------------------------------
            h1_t = st.tile([128, KC * 32], f32)
            transpose_1024(h1, h1_t)
            ps_f1 = pl.tile([32, FH], f32, tag="bank1", space="PSUM")
            for c in range(KC):
                w1t = sp.tile([128, FH], f32, tag="stream")
                nc.sync.dma_start(out=w1t[:], in_=w1k[128 * c : 128 * (c + 1), :])
                nc.tensor.matmul(
                    ps_f1[:, :],
                    lhsT=h1_t[:, 32 * c : 32 * (c + 1)],
                    rhs=w1t[:],
                    start=(c == 0), stop=(c == KC - 1),
                )
            b1f_t = st.tile([B, FH], f32)
            nc.sync.dma_start(out=b1f_t[:], in_=b1f_rep[:])
            u_pre = wp.tile([32, FH], f32)
            nc.vector.tensor_tensor(out=u_pre[:], in0=ps_f1[:], in1=b1f_t[:], op=AT.add)
            u = wp.tile([32, FH], f32)
            nc.scalar.activation(u[:], u_pre[:], AF.Gelu)
            u_t = st.tile([128, 4 * 32], f32)
            for c in range(4):
                pt = po.tile([128, 32], f32, tag="ptr", space="PSUM")
                nc.tensor.transpose(
                    out=pt[:, :],
                    in_=u[:, 128 * c : 128 * (c + 1)],
                    identity=ident[:32, :32],
                )
                nc.vector.tensor_copy(u_t[:, 32 * c : 32 * (c + 1)], pt[:, :])
            h2p = wp.tile([32, 1024], f32)
            for h in range(2):
                cs = slice(512 * h, 512 * (h + 1))
                ps_f2 = pb.tile([32, 512], f32, tag="big", space="PSUM")
                for c in range(4):
                    w2t = sp.tile([128, 512], f32, tag="stream")
                    nc.sync.dma_start(out=w2t[:], in_=w2k[128 * c : 128 * (c + 1), cs])
                    nc.tensor.matmul(
                        ps_f2[:, :],
                        lhsT=u_t[:, 32 * c : 32 * (c + 1)],
                        rhs=w2t[:],
                        start=(c == 0), stop=(c == 3),
                    )
                nc.vector.tensor_copy(h2p[:, cs], ps_f2[:])

            # ---- AllReduce #2: partial h2 --------------------------------
            cc2i = dp.tile([32, 1024], f32)
            cc2o = dp.tile([32, 1024], f32)
            nc.sync.dma_start(out=cc2i[:], in_=h2p[:])
            nc.gpsimd.collective_compute(
                "AllReduce",
                AT.add,
                replica_groups=[list(range(NC))],
                ins=[cc2i.opt()],
                outs=[cc2o.opt()],
            )
            h2s = wp.tile([32, 1024], f32)
            nc.sync.dma_start(out=h2s[:], in_=cc2o[:])
            b2f_t = st.tile([B, E], f32)
            nc.sync.dma_start(out=b2f_t[:], in_=b2f_rep[:])
            ln2_in = wp.tile([32, 1024], f32)
            nc.vector.tensor_tensor(out=ln2_in[:], in0=h2s[:], in1=b2f_t[:], op=AT.add)
            nc.vector.tensor_tensor(out=ln2_in[:], in0=ln2_in[:], in1=h1[:], op=AT.add)

            hf = layer_norm(ln2_in, g2r, b2r, "hf")

            # ---- logits (vocab shard) ------------------------------------
            hf_t = st.tile([128, KC * 32], f32)
            transpose_1024(hf, hf_t)
            NT = 8
            NW = VS // NT  # 500
            for n in range(NT):
                ps_lg = pl.tile([32, NW], f32, tag="bank1", space="PSUM")
                for c in range(KC):
                    tt = sp.tile([128, NW], f32, tag="stream")
                    nc.sync.dma_start(
                        out=tt[:],
                        in_=tembt[128 * c : 128 * (c + 1), NW * n : NW * (n + 1)],
                    )
                    nc.tensor.matmul(
                        ps_lg[:, :],
                        lhsT=hf_t[:, 32 * c : 32 * (c + 1)],
                        rhs=tt[:],
                        start=(c == 0), stop=(c == KC - 1),
                    )
                lg = lgp.tile([32, NW], f32, tag="lg")
                nc.vector.tensor_copy(lg[:], ps_lg[:])
                nc.sync.dma_start(out=logits[:, NW * n : NW * (n + 1)], in_=lg[:])

    _split_excess_waits(nc)
    return nc


_prog = None


def prepare_in_maps(x, tok_emb, pos_emb, qkv_w, qkv_b, in_w, in_b, pqc_params,
                    map_w, map_b, ffn_w1, ffn_b1, ffn_w2, ffn_b2,
                    ln1_g, ln1_b, ln2_g, ln2_b):
    x = np.asarray(x)
    tok_emb = np.ascontiguousarray(np.asarray(tok_emb, np.float32))
    pos_emb = np.asarray(pos_emb, np.float32)
    qkv_w = np.asarray(qkv_w, np.float32)
    qkv_b = np.asarray(qkv_b, np.float32)
    in_w = np.asarray(in_w, np.float32)
    in_b = np.asarray(in_b, np.float32)
    pqc_params = np.asarray(pqc_params, np.float64)
    map_w = np.asarray(map_w, np.float32)
    map_b = np.asarray(map_b, np.float32)
    ffn_w1 = np.asarray(ffn_w1, np.float32)
    ffn_b1 = np.asarray(ffn_b1, np.float32)
    ffn_w2 = np.asarray(ffn_w2, np.float32)
    ffn_b2 = np.asarray(ffn_b2, np.float32)
    ln1_g = np.asarray(ln1_g, np.float32)
    ln1_b = np.asarray(ln1_b, np.float32)
    ln2_g = np.asarray(ln2_g, np.float32)
    ln2_b = np.asarray(ln2_b, np.float32)

    # host folds (parameter-only)
    w_v = qkv_w[:, 2 * DQKV :]
    b_v = qkv_b[2 * DQKV :]
    W_a = (w_v @ in_w).astype(np.float32)                       # [E, Q]
    const_a = (pos_emb.mean(0) @ W_a + b_v @ in_w + in_b).astype(np.float32)

    UT = _build_ut(pqc_params)                                  # [DIM, DIM] cplx
    utr = np.ascontiguousarray(UT.real, np.float32)
    uti = np.ascontiguousarray(UT.imag, np.float32)

    ii = np.arange(DIM)
    bits = ((ii[:, None] >> (Q - 1 - np.arange(Q)[None, :])) & 1)  # [DIM, Q]
    S = (1 - 2 * bits).astype(np.float32)
    SM = np.ascontiguousarray(S @ map_w)                        # [DIM, E]
    sm_last = (map_b + pos_emb[-1]).astype(np.float32)[None, :]

    pc = bits.sum(1) % 4
    mr = np.choose(pc, [1.0, 0.0, -1.0, 0.0]).astype(np.float32)
    mi = np.choose(pc, [0.0, -1.0, 0.0, 1.0]).astype(np.float32)
    masks = np.empty((128, 3 * KC), np.float32)
    for c in range(KC):
        sl = slice(128 * c, 128 * (c + 1))
        masks[:, c] = mr[sl]
        masks[:, KC + c] = mi[sl]
        masks[:, 2 * KC + c] = -mi[sl]

    sel = np.zeros((128, 16 * 32), np.float32)
    for j in range(16):
        sel[:64, 32 * j + 2 * j] = 1.0
        sel[64:, 32 * j + 2 * j + 1] = 1.0

    rep = lambda v: np.ascontiguousarray(np.broadcast_to(v[None, :], (B, v.shape[0])), np.float32)
    g1r, b1r = rep(ln1_g), rep(ln1_b)
    g2r, b2r = rep(ln2_g), rep(ln2_b)
    b2f_rep = rep(ffn_b2)
    ca_rep = rep(const_a)

    xi = x.astype(np.int32)
    idx_l = np.ascontiguousarray(xi[:, -1:])                    # [B,1]

    in_maps = []
    for k in range(NC):
        idx_t = np.ascontiguousarray(
            xi[:, TS * k : TS * (k + 1)].reshape(B * TS, 1)
        )
        tembt = np.ascontiguousarray(tok_emb[VS * k : VS * (k + 1), :].T)
        w1k = np.ascontiguousarray(ffn_w1[:, FH * k : FH * (k + 1)])
        w2k = np.ascontiguousarray(ffn_w2[FH * k : FH * (k + 1), :])
        b1f_rep = rep(ffn_b1[FH * k : FH * (k + 1)])
        in_maps.append(dict(
            tok_emb=tok_emb, tembt=tembt, idx_t=idx_t, idx_l=idx_l, sel=sel,
            wa=W_a, ca_rep=ca_rep, masks=masks, utr=utr, uti=uti,
            sm=SM, sm_last=sm_last, g1r=g1r, b1r=b1r, g2r=g2r, b2r=b2r,
            b1f_rep=b1f_rep, b2f_rep=b2f_rep, w1k=w1k, w2k=w2k,
        ))
    return in_maps


def kernel(**inputs):
    global _prog
    in_maps = prepare_in_maps(**inputs)
    if _prog is None:
        _prog = _build_program()
    res = run_bass_kernel_spmd(
        _prog, in_maps, list(range(NC)), trace=TRACE,
    )
    _last["results"] = res
    out = np.concatenate([res.results[k]["logits"] for k in range(NC)], axis=1)
    return out, np.float32(0.0)
